# revision 1
# baseline (speedup 1.0000x reference)
"""AVFusion kernel for 8 trn2 NeuronCores — rank-9 factored FF layer 1.

Structure (per core, data-parallel over bs: 2 batches/core):
  All activations transposed (d on partitions as [128, d/128, tokens]).
  Math: the 2-way A/V softmax collapses to p = sigmoid((q.kA - q.kV)/sqrt(dk));
  x = vV + p*dV (per head chunk), so layer-1 preact is affine in the 8-dim p:
      y1pre[b,s,g] = y0[b,g] + C[b,g] @ p[b,s,g,:]
  with y0 = w1@vV + b1 and C[:,h] = w1[:,h-chunk] @ dV[h-chunk].
  Main loop: blocks of 8 g x 32 s = 256 tokens (order (b,g,s)). Layer 1 per
  m-chunk = ONE matmul with K=72: stationary rows (9*gl+h) hold C columns and
  row 9*gl+8 holds y0; the moving operand is block-diagonal [p-bands; ones].
  Then relu (ACT) and dense layer-2 (PE) + bias (DVE), output bf16 (host
  upcasts + untransposes). DMA issue is spread over sync/scalar/gpsimd.
"""

import numpy as np

BS, NSEG, NSEN, D, H, DK = 16, 64, 32, 1024, 8, 128
NCORES = 8
BPC = BS // NCORES           # batches per core = 2
TOK_AV = BPC * NSEG          # 128
TOK_S = BPC * NSEN           # 64
TOK_OUT = BPC * NSEN * NSEG  # 4096
KC = D // 128                # 8 d-chunks
GC = 8                       # g's per block
RG = H + 1                   # rows per g in the stationary (8 C + 1 y0)
KB = GC * RG                 # contraction rows per block = 72
BLK = GC * NSEN              # 256 tokens per block
NBLK = TOK_OUT // BLK        # 16 blocks per core
SCALE = 1.0 / np.sqrt(np.float32(DK))

_CACHE = {}


def _build_nc():
    import concourse.bass as bass
    import concourse.mybir as mybir
    import concourse.tile as tile
    from concourse import bacc
    from contextlib import ExitStack

    fp32 = mybir.dt.float32
    bf16 = mybir.dt.bfloat16
    AF = mybir.ActivationFunctionType
    ALU = mybir.AluOpType

    nc = bacc.Bacc(None, target_bir_lowering=False)

    # ---- per-core DRAM I/O (host pre-transposed layouts) ----
    AT = nc.dram_tensor("AT", [128, KC, TOK_AV], bf16, kind="ExternalInput")
    VT = nc.dram_tensor("VT", [128, KC, TOK_AV], bf16, kind="ExternalInput")
    ST = nc.dram_tensor("ST", [128, KC, TOK_S], bf16, kind="ExternalInput")
    wts = {}
    for w in ["wA", "wV", "wS", "wq", "wk", "wv", "w1", "w2"]:
        wts[w] = nc.dram_tensor(w, [128, KC, D], bf16, kind="ExternalInput")
    bias = {}
    for b in ["bA", "bV", "bS", "bq", "bv", "b2"]:
        bias[b] = nc.dram_tensor(b, [128, KC], fp32, kind="ExternalInput")
    B1R = nc.dram_tensor("b1r", [1, D], bf16, kind="ExternalInput")
    PCONST = nc.dram_tensor("pconst", [KB, BLK], bf16, kind="ExternalInput")
    OUT = nc.dram_tensor("OUT", [128, KC, TOK_OUT], bf16,
                         kind="ExternalOutput")

    with tile.TileContext(nc) as tc, ExitStack() as ctx:
        pers = ctx.enter_context(tc.tile_pool(name="pers", bufs=1))
        bsb = {}
        for b in bias:
            bsb[b] = pers.tile([128, KC], fp32, tag=f"b_{b}", name=f"b_{b}")
            nc.sync.dma_start(bsb[b][:], bias[b][:])
        b1r = pers.tile([1, D], bf16, tag="b1r", name="b1r")
        nc.sync.dma_start(b1r[:], B1R[:])
        ones1 = pers.tile([1, TOK_AV], bf16, tag="ones1", name="ones1")
        nc.vector.memset(ones1[:], 1.0)

        # stationary + rhs tiles for the main loop (persist)
        Ccat = [pers.tile([KB, D], bf16, tag=f"Ccat{i}", name=f"Ccat{i}")
                for i in range(NBLK)]
        pblk = [pers.tile([KB, BLK], bf16, tag=f"pblk{i}", name=f"pblk{i}")
                for i in range(NBLK)]
        for i in range(NBLK):
            nc.gpsimd.dma_start(pblk[i][:], PCONST[:])  # zeros + ones rows

        wm = ctx.enter_context(tc.tile_pool(name="wm", bufs=1))
        w1sb = wm.tile([128, KC, D], bf16, tag="w1", name="w1")
        w2sb = wm.tile([128, KC, D], bf16, tag="w2", name="w2")

        _eng = [nc.sync, nc.scalar, nc.gpsimd]
        _rr = [0]

        def dma_rr(dst, src):
            e = _eng[_rr[0] % len(_eng)]
            _rr[0] += 1
            e.dma_start(dst, src)

        # ---------------- prologue ----------------
        with tc.tile_pool(name="wp", bufs=2) as wp, \
             tc.tile_pool(name="acts", bufs=1) as acts, \
             tc.tile_pool(name="ppsum", bufs=2, space="PSUM") as ppsum, \
             tc.tile_pool(name="cpsum", bufs=2, space="PSUM") as cpsum, \
             tc.tile_pool(name="lpsum", bufs=2, space="PSUM") as lpsum, \
             tc.tile_pool(name="ctp", bufs=9) as ctp, \
             tc.tile_pool(name="ptmp", bufs=2) as ptmp:

            ATs = acts.tile([128, KC, TOK_AV], bf16, tag="ATs", name="ATs")
            VTs = acts.tile([128, KC, TOK_AV], bf16, tag="VTs", name="VTs")
            STs = acts.tile([128, KC, TOK_S], bf16, tag="STs", name="STs")

            def wtile(wname):
                t = wp.tile([128, KC, D], bf16, tag="w", name="w")
                dma_rr(t[:], wts[wname][:])
                return t

            def proj(dst, wt_sb, src, bias_tile):
                """dst[:, m, :] = w @ src (+ b)   (transposed layout)."""
                ntok = src.shape[2]
                for m in range(KC):
                    ps = ppsum.tile([128, 512], fp32, tag="proj_ps",
                                    name="proj_ps")
                    for k in range(KC):
                        nc.tensor.matmul(
                            ps[:, :ntok],
                            wt_sb[:, k, m * 128:(m + 1) * 128],
                            src[:, k, :],
                            start=(k == 0), stop=(k == KC - 1),
                        )
                    if bias_tile is None:
                        nc.vector.tensor_copy(dst[:, m, :], ps[:, :ntok])
                    else:
                        nc.vector.tensor_tensor(
                            dst[:, m, :], ps[:, :ntok],
                            bias_tile[:, m, None].to_broadcast([128, ntok]),
                            ALU.add,
                        )

            A2T = acts.tile([128, KC, TOK_AV], bf16, tag="A2T", name="A2T")
            V2T = acts.tile([128, KC, TOK_AV], bf16, tag="V2T", name="V2T")
            dAVT = acts.tile([128, KC, TOK_AV], bf16, tag="dAVT", name="dAVT")
            S2T = acts.tile([128, KC, TOK_S], bf16, tag="S2T", name="S2T")
            dkT = acts.tile([128, KC, TOK_AV], bf16, tag="dkT", name="dkT")
            qT = acts.tile([128, KC, TOK_S], bf16, tag="qT", name="qT")
            vVT = acts.tile([128, KC, TOK_AV], bf16, tag="vVT", name="vVT")
            dVT = acts.tile([128, KC, TOK_AV], bf16, tag="dVT", name="dVT")

            # attention path first: its serial tail (128 p-scatter DMAs)
            # is longest, and it only needs wA/wV/wS/wk/wq.
            wAs = wtile("wA")
            dma_rr(ATs[:], AT[:])
            dma_rr(VTs[:], VT[:])
            dma_rr(STs[:], ST[:])
            proj(A2T, wAs, ATs, bsb["bA"])
            wVs = wtile("wV")
            proj(V2T, wVs, VTs, bsb["bV"])
            for m in range(KC):
                nc.vector.tensor_tensor(dAVT[:, m, :], A2T[:, m, :],
                                        V2T[:, m, :], ALU.subtract)
            wSs = wtile("wS")
            dma_rr(w1sb[:], wts["w1"][:])
            proj(S2T, wSs, STs, bsb["bS"])
            wks = wtile("wk")
            proj(dkT, wks, dAVT, None)          # bk cancels in kA - kV
            wqs = wtile("wq")
            proj(qT, wqs, S2T, bsb["bq"])
            wvs = wtile("wv")
            dma_rr(w2sb[:], wts["w2"][:])

            # logits -> p = sigmoid(scale*(q . dk)) laid out [64 g, (h,s)],
            # then scatter into per-block block-diagonal rhs tiles.
            for b in range(BPC):
                pgs = ptmp.tile([64, H * NSEN], bf16, tag="pgs", name="pgs")
                for h in range(H):
                    lgp = lpsum.tile([64, NSEN], fp32, tag="lgp", name="lgp")
                    nc.tensor.matmul(
                        lgp[:],
                        dkT[:, h, b * NSEG:(b + 1) * NSEG],
                        qT[:, h, b * NSEN:(b + 1) * NSEN],
                        start=True, stop=True)
                    nc.scalar.activation(pgs[:, h * NSEN:(h + 1) * NSEN],
                                         lgp[:], AF.Sigmoid,
                                         scale=float(SCALE))
                for gc in range(NBLK // BPC):
                    i = b * (NBLK // BPC) + gc
                    for gl in range(GC):
                        g = gc * GC + gl
                        dma_rr(
                            pblk[i][RG * gl:RG * gl + H,
                                    32 * gl:32 * gl + 32],
                            pgs[g:g + 1, :])

            # v path + C/y0 build (needs wv, w1)
            proj(vVT, wvs, V2T, bsb["bv"])
            proj(dVT, wvs, dAVT, None)          # bv cancels in vA - vV

            # C: Ctmp_h[(b,g), d] = (w1 @ dV_hchunk)[d] (already transposed
            # for stationary use); y0[(b,g), d] = (w1 @ vV)[d] + b1[d].
            ctmps = []
            for h in range(H):
                psc = cpsum.tile([128, D], fp32, tag="psc", name="psc")
                for m in range(KC):
                    nc.tensor.matmul(
                        psc[:, m * 128:(m + 1) * 128],
                        dVT[:, h, :],
                        w1sb[:, h, m * 128:(m + 1) * 128],
                        start=True, stop=True)
                ctmp = ctp.tile([128, D], bf16, tag="ctmp", name="ctmp")
                nc.vector.tensor_copy(ctmp[:], psc[:])
                ctmps.append(ctmp)
            psy = cpsum.tile([128, D], fp32, tag="psc", name="psy")
            for m in range(KC):
                for k in range(KC):
                    nc.tensor.matmul(
                        psy[:, m * 128:(m + 1) * 128],
                        vVT[:, k, :],
                        w1sb[:, k, m * 128:(m + 1) * 128],
                        start=(k == 0), stop=False)
                nc.tensor.matmul(
                    psy[:, m * 128:(m + 1) * 128],
                    ones1[:],
                    b1r[:, m * 128:(m + 1) * 128],
                    start=False, stop=True)
            y0tmp = ctp.tile([128, D], bf16, tag="ctmp", name="y0tmp")
            nc.vector.tensor_copy(y0tmp[:], psy[:])

            # scatter C/y0 into stationaries, block-major so block 0 is
            # ready first
            for b in range(BPC):
                for gc in range(NBLK // BPC):
                    i = b * (NBLK // BPC) + gc
                    g0 = b * NSEG + gc * GC
                    for h in range(H):
                        dma_rr(Ccat[i][h:KB:RG, :], ctmps[h][g0:g0 + GC, :])
                    dma_rr(Ccat[i][H:KB:RG, :], y0tmp[g0:g0 + GC, :])

        # ---------------- main loop ----------------
        with tc.tile_pool(name="blk", bufs=3) as blkp, \
             tc.tile_pool(name="f1ps", bufs=4, space="PSUM") as f1ps, \
             tc.tile_pool(name="f2ps", bufs=4, space="PSUM") as f2ps:

            for i in range(NBLK):
                tok0 = i * BLK
                y1 = blkp.tile([128, KC, BLK], bf16, tag="y1blk",
                               name="y1blk")
                for m in range(KC):
                    ps = f1ps.tile([128, BLK], fp32, tag="f1", name="f1")
                    nc.tensor.matmul(ps[:], Ccat[i][:, m * 128:(m + 1) * 128],
                                     pblk[i][:], start=True, stop=True)
                    nc.scalar.activation(y1[:, m, :], ps[:], AF.Relu)
                ob = blkp.tile([128, KC, BLK], bf16, tag="oblk", name="oblk")
                for m in range(KC):
                    ps = f2ps.tile([128, BLK], fp32, tag="f2", name="f2")
                    for k in range(KC):
                        nc.tensor.matmul(ps[:],
                                         w2sb[:, k, m * 128:(m + 1) * 128],
                                         y1[:, k, :],
                                         start=(k == 0), stop=(k == KC - 1))
                    nc.vector.tensor_tensor(
                        ob[:, m, :], ps[:],
                        bsb["b2"][:, m, None].to_broadcast([128, BLK]),
                        ALU.add)
                nc.sync.dma_start(OUT[:, :, tok0:tok0 + BLK], ob[:])

    nc.finalize()
    return nc


def _prep_core_inputs(inputs, core):
    b0 = core * BPC
    f32 = np.float32

    import ml_dtypes
    bf16 = ml_dtypes.bfloat16

    def t_act(x, ntok):
        # (bpc, n, D) -> [128, KC, ntok] with token = (b, n)
        flat = np.ascontiguousarray(x[b0:b0 + BPC]).reshape(ntok, KC, 128)
        return np.ascontiguousarray(flat.transpose(2, 1, 0)).astype(bf16)

    def t_w(w, dtype=f32):
        # (D, D) -> w.T as [128, KC, D]:  [p, k, n] = w[n, k*128+p]
        wt = np.ascontiguousarray(w.T).reshape(KC, 128, D)
        return np.ascontiguousarray(wt.transpose(1, 0, 2)).astype(dtype)

    def t_b(b):
        return np.ascontiguousarray(b.reshape(KC, 128).T, dtype=f32)

    m = {
        "AT": t_act(inputs["A"], TOK_AV),
        "VT": t_act(inputs["V"], TOK_AV),
        "ST": t_act(inputs["S"], TOK_S),
    }
    for w in ["wA", "wV", "wS", "wq", "wk", "wv", "w1", "w2"]:
        m[w] = t_w(inputs[w], bf16)
    for b in ["bA", "bV", "bS", "bq", "bv", "b2"]:
        m[b] = t_b(inputs[b])
    m["b1r"] = np.ascontiguousarray(inputs["b1"].reshape(1, D)).astype(bf16)
    pc = np.zeros((KB, BLK), dtype=bf16)
    for gl in range(GC):
        pc[RG * gl + H, 32 * gl:32 * gl + 32] = 1.0
    m["pconst"] = pc
    return m


def kernel(**inputs):
    import os
    from concourse.bass_utils import run_bass_kernel_spmd

    inputs = {k: np.asarray(v, dtype=np.float32) for k, v in inputs.items()}
    if "nc" not in _CACHE:
        _CACHE["nc"] = _build_nc()
    nc = _CACHE["nc"]

    in_maps = [_prep_core_inputs(inputs, c) for c in range(NCORES)]
    trace = os.environ.get("TRACE", "0") == "1"
    res = run_bass_kernel_spmd(nc, in_maps, core_ids=list(range(NCORES)),
                               trace=trace)
    _CACHE["last_results"] = res

    out = np.empty((BS, NSEN, NSEG, D), dtype=np.float32)
    for c in range(NCORES):
        oc = res.results[c]["OUT"].astype(np.float32)  # tok = (b, g, s)
        oc = oc.reshape(128, KC, BPC, NSEG, NSEN).transpose(2, 4, 3, 1, 0)
        out[c * BPC:(c + 1) * BPC] = oc.reshape(BPC, NSEN, NSEG, D)
    return out



# revision 6
# speedup vs baseline: 1.1843x; 1.1843x over previous
"""AVFusion kernel for 8 trn2 NeuronCores — v2.

Per core (data-parallel over bs, 2 batches/core), all activations
transposed (d on partitions as [128, d/128, tokens]).

Math: the 2-way A/V softmax collapses to p = sigmoid((q.kA - q.kV)/sqrt(dk));
x = vV + p*dV per head chunk, so the layer-1 preact is affine in the 8-dim p:
    y1pre[b,s,g] = y0[b,g] + C[b,g] @ p[b,s,g,:]
with y0 = w1@vV + b1 and C[:,h] = w1[:,h-chunk] @ dV[h-chunk].

v2 changes vs v1:
  * Host-side weight folding: wkA=wk@wA, wkVn=-wk@wV, wqS=wq@wS,
    wvA=wv@wA, wvVn=-wv@wV (+ folded bias constants) -> projections become
    single PSUM accumulation chains, 7 device weight matrices instead of 8.
  * AV tokens permuted as tok = b*64 + gl*8 + gc (g = gc*8+gl), which makes
    the C/y0 scatter into the layer-1 stationaries 18+2 strided DMAs
    (one per (h,b)) instead of 144 tiny ones.
  * Weight DMAs split into per-m-chunk pieces issued in consumption order
    on the gpsimd queue; attention path computes per-head as chunks land.
  * p scatter: 64 DMAs (one per (gl,gc), both batches + all heads at once),
    split across the sync and scalar queues.
  * Main loop software-pipelined one block deep (L1(i+1) emitted before
    L2(i)) so relu drains never stall the PE; layer-1 PSUM packs 2 m-chunks
    per bank -> half the relu ACT ops.
"""

import numpy as np

BS, NSEG, NSEN, D, H, DK = 16, 64, 32, 1024, 8, 128
NCORES = 8
BPC = BS // NCORES           # batches per core = 2
TOK_AV = BPC * NSEG          # 128
TOK_S = BPC * NSEN           # 64
TOK_OUT = BPC * NSEN * NSEG  # 4096
KC = D // 128                # 8 d-chunks
GC = 8                       # g's per block
RG = H + 1                   # rows per g in the stationary (8 C + 1 y0)
KB = GC * RG                 # contraction rows per block = 72
BLK = GC * NSEN              # 256 tokens per block
NBLK = TOK_OUT // BLK        # 16 blocks per core
SCALE = 1.0 / np.sqrt(np.float32(DK))

_CACHE = {}

# cst rows
I_CK, I_CD, I_CQ, I_CV, I_B2 = range(5)


def _build_nc():
    import concourse.bass as bass
    import concourse.mybir as mybir
    import concourse.tile as tile
    from concourse import bacc
    from contextlib import ExitStack

    fp32 = mybir.dt.float32
    bf16 = mybir.dt.bfloat16
    AF = mybir.ActivationFunctionType
    ALU = mybir.AluOpType

    nc = bacc.Bacc(None, target_bir_lowering=False)

    # ---- per-core DRAM I/O (host pre-transposed / pre-folded layouts) ----
    AT = nc.dram_tensor("AT", [128, KC, TOK_AV], bf16, kind="ExternalInput")
    VT = nc.dram_tensor("VT", [128, KC, TOK_AV], bf16, kind="ExternalInput")
    ST = nc.dram_tensor("ST", [128, KC, TOK_S], bf16, kind="ExternalInput")
    WNAMES = ["wkA", "wkVn", "wqS", "wvVn", "wvA", "w1f", "w2f"]
    wts = {w: nc.dram_tensor(w, [128, KC, D], bf16, kind="ExternalInput")
           for w in WNAMES}
    CST = nc.dram_tensor("cst", [128, 5, KC], fp32, kind="ExternalInput")
    B1R = nc.dram_tensor("b1r", [1, D], bf16, kind="ExternalInput")
    PCONST = nc.dram_tensor("pconst", [KB, BPC, GC, BLK], bf16,
                            kind="ExternalInput")
    OUT = nc.dram_tensor("OUT", [128, KC, TOK_OUT], bf16,
                         kind="ExternalOutput")

    with tile.TileContext(nc) as tc, ExitStack() as ctx:
        pers = ctx.enter_context(tc.tile_pool(name="pers", bufs=1))
        cst = pers.tile([128, 5, KC], fp32, tag="cst", name="cst")
        b1r = pers.tile([1, D], bf16, tag="b1r", name="b1r")
        ones1 = pers.tile([1, TOK_AV], bf16, tag="ones1", name="ones1")
        pall = pers.tile([KB, BPC, GC, BLK], bf16, tag="pall", name="pall")
        Ccat = pers.tile([KB, NBLK * D], bf16, tag="Ccat", name="Ccat")
        w2sb = pers.tile([128, KC, D], bf16, tag="w2sb", name="w2sb")

        nc.vector.memset(ones1[:], 1.0)

        # early small loads on the gpsimd (weight) queue
        nc.gpsimd.dma_start(cst[:], CST[:])
        nc.gpsimd.dma_start(b1r[:], B1R[:])
        nc.gpsimd.dma_start(pall[:], PCONST[:])

        def csc(row, m):
            # per-partition scalar AP for output chunk m
            return cst[:, row, m:m + 1]

        with tc.tile_pool(name="wpro", bufs=1) as wpro, \
             tc.tile_pool(name="acts", bufs=1) as acts, \
             tc.tile_pool(name="ppsum", bufs=3, space="PSUM") as ppsum, \
             tc.tile_pool(name="cpsum", bufs=2, space="PSUM") as cpsum, \
             tc.tile_pool(name="ypsum", bufs=1, space="PSUM") as ypsum:

            ATs = acts.tile([128, KC, TOK_AV], bf16, tag="ATs", name="ATs")
            VTs = acts.tile([128, KC, TOK_AV], bf16, tag="VTs", name="VTs")
            STs = acts.tile([128, KC, TOK_S], bf16, tag="STs", name="STs")
            nc.gpsimd.dma_start(ATs[:], AT[:])
            nc.gpsimd.dma_start(VTs[:], VT[:])
            nc.gpsimd.dma_start(STs[:], ST[:])

            wsb = {w: wpro.tile([128, KC, D], bf16, tag=w, name=w)
                   for w in WNAMES[:-1]}
            wsb["w2f"] = w2sb

            # ---- weight chunk DMAs in consumption order (gpsimd queue) ----
            def wchunk(w, m):
                sl = slice(m * 128, (m + 1) * 128)
                nc.gpsimd.dma_start(wsb[w][:, :, sl], wts[w][:, :, sl])

            for h in range(H):
                wchunk("wkA", h)
                wchunk("wkVn", h)
                wchunk("wqS", h)
            for m in range(KC):
                wchunk("wvVn", m)
                wchunk("wvA", m)
            for m in range(KC):
                wchunk("w1f", m)
            for m in range(KC):
                wchunk("w2f", m)

            dkT = acts.tile([128, KC, TOK_AV], bf16, tag="dkT", name="dkT")
            qT = acts.tile([128, KC, TOK_S], bf16, tag="qT", name="qT")
            vVT = acts.tile([128, KC, TOK_AV], bf16, tag="vVT", name="vVT")
            dVT = acts.tile([128, KC, TOK_AV], bf16, tag="dVT", name="dVT")
            ctall = acts.tile([128, H, D], bf16, tag="ctall", name="ctall")
            y0tmp = acts.tile([128, D], bf16, tag="y0tmp", name="y0tmp")
            pgs = acts.tile([64, H, BPC, NSEN], bf16, tag="pgs", name="pgs")

            # ---- attention path, per head h (chunk h of dk and q) ----
            for h in range(H):
                sl = slice(h * 128, (h + 1) * 128)
                ps = ppsum.tile([128, TOK_AV], fp32, tag="pp", name="pp")
                for k in range(KC):
                    nc.tensor.matmul(ps[:], wsb["wkA"][:, k, sl],
                                     ATs[:, k, :], start=(k == 0), stop=False)
                for k in range(KC):
                    nc.tensor.matmul(ps[:], wsb["wkVn"][:, k, sl],
                                     VTs[:, k, :], start=False,
                                     stop=(k == KC - 1))
                nc.vector.tensor_scalar_add(dkT[:, h, :], ps[:],
                                            csc(I_CK, h))
                ps2 = ppsum.tile([128, TOK_AV], fp32, tag="pp", name="pp2")
                for k in range(KC):
                    nc.tensor.matmul(ps2[:, :TOK_S], wsb["wqS"][:, k, sl],
                                     STs[:, k, :], start=(k == 0),
                                     stop=(k == KC - 1))
                nc.vector.tensor_scalar_add(qT[:, h, :], ps2[:, :TOK_S],
                                            csc(I_CQ, h))
                for b in range(BPC):
                    lgp = ppsum.tile([64, NSEN], fp32, tag="pp", name="lgp")
                    nc.tensor.matmul(
                        lgp[:],
                        dkT[:, h, b * NSEG:(b + 1) * NSEG],
                        qT[:, h, b * NSEN:(b + 1) * NSEN],
                        start=True, stop=True)
                    nc.scalar.activation(pgs[:, h, b, :], lgp[:], AF.Sigmoid,
                                         scale=float(SCALE))

            # ---- p scatter: one DMA per (gl, gc), sync/scalar queues ----
            for gc in range(GC):
                eng = nc.sync if gc < 4 else nc.scalar
                for gl in range(GC):
                    j = gl * 8 + gc
                    eng.dma_start(
                        pall[RG * gl:RG * gl + H, :, gc,
                             32 * gl:32 * gl + 32],
                        pgs[j:j + 1, :, :, :])

            # ---- v path per m-chunk ----
            for m in range(KC):
                sl = slice(m * 128, (m + 1) * 128)
                ps = ppsum.tile([128, TOK_AV], fp32, tag="pp", name="ppv")
                for k in range(KC):
                    nc.tensor.matmul(ps[:], wsb["wvVn"][:, k, sl],
                                     VTs[:, k, :], start=(k == 0),
                                     stop=(k == KC - 1))
                nc.vector.tensor_scalar(vVT[:, m, :], ps[:], -1.0,
                                        csc(I_CV, m), ALU.mult, ALU.add)
                ps2 = ppsum.tile([128, TOK_AV], fp32, tag="pp", name="ppd")
                for k in range(KC):
                    nc.tensor.matmul(ps2[:], wsb["wvA"][:, k, sl],
                                     ATs[:, k, :], start=(k == 0), stop=False)
                for k in range(KC):
                    nc.tensor.matmul(ps2[:], wsb["wvVn"][:, k, sl],
                                     VTs[:, k, :], start=False,
                                     stop=(k == KC - 1))
                nc.vector.tensor_scalar_add(dVT[:, m, :], ps2[:],
                                            csc(I_CD, m))

            # ---- C / y0 build, m-major (consumes w1 chunks as they land) --
            psy = ypsum.tile([128, D], fp32, tag="psy", name="psy")
            for m in range(KC):
                sl = slice(m * 128, (m + 1) * 128)
                for hh in range(2):
                    cps = cpsum.tile([128, D // 2], fp32, tag="cps",
                                     name="cps")
                    for h4 in range(4):
                        h = hh * 4 + h4
                        nc.tensor.matmul(cps[:, h4 * 128:(h4 + 1) * 128],
                                         dVT[:, h, :], wsb["w1f"][:, h, sl],
                                         start=True, stop=True)
                    nc.vector.tensor_copy(ctall[:, hh * 4:hh * 4 + 4, sl],
                                          cps[:])
                for k in range(KC):
                    nc.tensor.matmul(psy[:, sl], vVT[:, k, :],
                                     wsb["w1f"][:, k, sl],
                                     start=(k == 0), stop=False)
                nc.tensor.matmul(psy[:, sl], ones1[:], b1r[:, sl],
                                 start=False, stop=True)
            nc.vector.tensor_copy(y0tmp[:], psy[:])

            # ---- C / y0 scatter into layer-1 stationaries (sync queue) ----
            for b in range(BPC):
                cs = slice(b * NBLK // BPC * D, (b + 1) * NBLK // BPC * D)
                for h in range(H):
                    nc.sync.dma_start(Ccat[h:KB:RG, cs],
                                      ctall[b * 64:(b + 1) * 64, h, :])
                nc.sync.dma_start(Ccat[H:KB:RG, cs],
                                  y0tmp[b * 64:(b + 1) * 64, :])

        # ---------------- main loop (pipelined one block deep) ----------
        with tc.tile_pool(name="y1p", bufs=3) as y1p, \
             tc.tile_pool(name="obp", bufs=3) as obp, \
             tc.tile_pool(name="f1ps", bufs=4, space="PSUM") as f1ps, \
             tc.tile_pool(name="f2ps", bufs=4, space="PSUM") as f2ps:

            y1s = [None] * NBLK

            def emit_l1(i):
                b, gc = divmod(i, GC)
                y1 = y1p.tile([128, KC, BLK], bf16, tag="y1", name="y1")
                y1s[i] = y1
                for mp in range(KC // 2):
                    ps = f1ps.tile([128, 2 * BLK], fp32, tag="f1", name="f1")
                    for half in range(2):
                        m = 2 * mp + half
                        nc.tensor.matmul(
                            ps[:, half * BLK:(half + 1) * BLK],
                            Ccat[:, i * D + m * 128:i * D + (m + 1) * 128],
                            pall[:, b, gc, :], start=True, stop=True)
                    nc.scalar.activation(y1[:, 2 * mp:2 * mp + 2, :], ps[:],
                                         AF.Relu)

            def emit_l2(i):
                y1 = y1s[i]
                ob = obp.tile([128, KC, BLK], bf16, tag="ob", name="ob")
                for m in range(KC):
                    sl = slice(m * 128, (m + 1) * 128)
                    ps = f2ps.tile([128, BLK], fp32, tag="f2", name="f2")
                    for k in range(KC):
                        nc.tensor.matmul(ps[:], w2sb[:, k, sl], y1[:, k, :],
                                         start=(k == 0), stop=(k == KC - 1))
                    nc.vector.tensor_scalar_add(ob[:, m, :], ps[:],
                                                csc(I_B2, m))
                y1s[i] = None
                nc.gpsimd.dma_start(OUT[:, :, i * BLK:(i + 1) * BLK], ob[:])

            emit_l1(0)
            for i in range(1, NBLK):
                emit_l1(i)
                emit_l2(i - 1)
            emit_l2(NBLK - 1)

    nc.finalize()
    return nc


def _prep_core_inputs(inputs, core, folded):
    b0 = core * BPC
    f32 = np.float32

    import ml_dtypes
    bf16 = ml_dtypes.bfloat16

    # AV token permutation: position b*64 + gl*8 + gc holds (b, g=gc*8+gl)
    pos = np.arange(TOK_AV)
    pb = pos // 64
    pr = pos % 64
    pg = (pr % 8) * 8 + pr // 8          # g = gc*8+gl with gl=pr//8, gc=pr%8

    def t_act_av(x):
        xp = x[b0 + pb, pg]              # (128, D) in pi order
        flat = xp.reshape(TOK_AV, KC, 128)
        return np.ascontiguousarray(flat.transpose(2, 1, 0)).astype(bf16)

    def t_act_s(x):
        flat = np.ascontiguousarray(x[b0:b0 + BPC]).reshape(TOK_S, KC, 128)
        return np.ascontiguousarray(flat.transpose(2, 1, 0)).astype(bf16)

    m = {
        "AT": t_act_av(inputs["A"]),
        "VT": t_act_av(inputs["V"]),
        "ST": t_act_s(inputs["S"]),
    }
    m.update(folded)
    return m


def _prep_folded(inputs):
    """Core-independent folded weights/constants (computed once)."""
    f32 = np.float32
    import ml_dtypes
    bf16 = ml_dtypes.bfloat16

    def t_w(w):
        wt = np.ascontiguousarray(np.asarray(w, f32).T).reshape(KC, 128, D)
        return np.ascontiguousarray(wt.transpose(1, 0, 2)).astype(bf16)

    def t_b(b):
        return np.ascontiguousarray(np.asarray(b, f32).reshape(KC, 128).T,
                                    dtype=f32)

    wA, wV, wS = inputs["wA"], inputs["wV"], inputs["wS"]
    wq, wk, wv = inputs["wq"], inputs["wk"], inputs["wv"]
    w1, w2 = inputs["w1"], inputs["w2"]
    bA, bV, bS = inputs["bA"], inputs["bV"], inputs["bS"]
    bq, bv = inputs["bq"], inputs["bv"]
    b1, b2 = inputs["b1"], inputs["b2"]

    m = {
        "wkA": t_w(wk @ wA),
        "wkVn": t_w(-(wk @ wV)),
        "wqS": t_w(wq @ wS),
        "wvVn": t_w(-(wv @ wV)),
        "wvA": t_w(wv @ wA),
        "w1f": t_w(w1),
        "w2f": t_w(w2),
    }
    cst = np.stack([
        t_b(wk @ (bA - bV)),
        t_b(wv @ (bA - bV)),
        t_b(wq @ bS + bq),
        t_b(wv @ bV + bv),
        t_b(b2),
    ], axis=1)                           # [128, 5, KC]
    m["cst"] = np.ascontiguousarray(cst)
    m["b1r"] = np.ascontiguousarray(b1.reshape(1, D)).astype(bf16)
    pc = np.zeros((KB, BPC, GC, BLK), dtype=bf16)
    for gl in range(GC):
        pc[RG * gl + H, :, :, 32 * gl:32 * gl + 32] = 1.0
    m["pconst"] = pc
    return m


def kernel(**inputs):
    import os
    from concourse.bass_utils import run_bass_kernel_spmd

    inputs = {k: np.asarray(v, dtype=np.float32) for k, v in inputs.items()}
    if "nc" not in _CACHE:
        _CACHE["nc"] = _build_nc()
    nc = _CACHE["nc"]

    folded = _prep_folded(inputs)
    in_maps = [_prep_core_inputs(inputs, c, folded) for c in range(NCORES)]
    trace = os.environ.get("TRACE", "0") == "1"
    res = run_bass_kernel_spmd(nc, in_maps, core_ids=list(range(NCORES)),
                               trace=trace)
    _CACHE["last_results"] = res

    out = np.empty((BS, NSEN, NSEG, D), dtype=np.float32)
    for c in range(NCORES):
        oc = res.results[c]["OUT"].astype(np.float32)
        # tok = (b, gc, gl, s); g = gc*8 + gl
        oc = oc.reshape(128, KC, BPC, GC, GC, NSEN)
        oc = oc.transpose(2, 5, 3, 4, 1, 0)    # (b, s, gc, gl, k, dd)
        out[c * BPC:(c + 1) * BPC] = oc.reshape(BPC, NSEN, NSEG, D)
    return out
